# revision 15
# baseline (speedup 1.0000x reference)
"""Trainium2 Bass kernel for ComputeRealAngleInput (gnn angle descriptor).

Contract: kernel(**inputs) takes the FULL unsharded inputs (as produced by
setup_inputs()) and returns the full output tuple
(atom_i_idx.reshape(-1), ang_desc[C, N*(N-1), 3+3F]).

Strategy (data-parallel over center atoms, 8 NeuronCores):
  - shard atom_i_idx / atom_j_idx / dist_ij along dim 0 (500 centers/core)
  - replicate atoms_xyz / atoms_long / embed_table on every core
  - per core: 4 tiles of 125 centers (partition dim = centers)
      * indirect-DMA gather of xyz rows + species ids (17 per center)
      * embeddings via one-hot(species)*scale -> PE transpose -> matmul
        with a block-diagonal embed table (computes emb/d for all slots)
      * pairwise angles via broadcast-AP vector ops;
        arccos(x) = pi/2 - arctan(x * rsqrt(1 - x^2))
      * output assembled in SBUF as the full 16x16 (j,k) grid; the DMA to
        DRAM skips the diagonal rows with the flat[1:].reshape(15,17)[:,:16]
        access-pattern trick so exactly 240 rows per center are written.
"""

import numpy as np

import concourse.bass as bass
import concourse.bacc as bacc
import concourse.mybir as mybir
from concourse import tile
from concourse.bass_utils import run_bass_kernel_spmd
from concourse.masks import make_identity

# ---- problem constants (hardcoded; must match the reference problem) ----
N_CORES = 8
C_FULL = 4000
CP = C_FULL // N_CORES  # 500 centers per core
P = 125                 # centers per tile (partition dim)
NT = CP // P            # 4 center tiles per core
N = 16                  # neighbours
F = 64                  # embedding features
NA = 20000              # atoms
NS = 8                  # species
ROW = 3 + 3 * F         # 195 output columns
NPAIR = N * (N - 1)     # 240 off-diagonal (j,k) pairs
COS_EPS = 1e-8
CLIP = 0.9999

FP32 = mybir.dt.float32
I32 = mybir.dt.int32
AF = mybir.ActivationFunctionType
ALU = mybir.AluOpType


def build_program(cp: int = CP) -> bass.Bass:
    nt = cp // P
    nc = bacc.Bacc("TRN2", target_bir_lowering=False, debug=False)

    ij_idx = nc.dram_tensor("ij_idx", [cp, 17], I32, kind="ExternalInput")
    dist = nc.dram_tensor("dist", [cp, N], FP32, kind="ExternalInput")
    axyz = nc.dram_tensor("axyz", [NA, 3], FP32, kind="ExternalInput")
    along = nc.dram_tensor("along", [NA, 2], I32, kind="ExternalInput")
    etab = nc.dram_tensor("etab", [NS, F], FP32, kind="ExternalInput")
    out = nc.dram_tensor("out", [cp, NPAIR * ROW], FP32, kind="ExternalOutput")

    od = out.ap()

    with tile.TileContext(nc) as tc:
        with (
            tc.tile_pool(name="consts", bufs=1) as consts,
            tc.tile_pool(name="io", bufs=4) as io,
            tc.tile_pool(name="work", bufs=2) as work,
            tc.tile_pool(name="ochp", bufs=2) as ochp,
            tc.tile_pool(name="psp", bufs=2, space="PSUM") as psp,
        ):
            # ---- one-time constants ----
            ident = consts.tile([128, 128], FP32)
            make_identity(nc, ident[:])

            iota_i = consts.tile([128, NS], I32)
            nc.gpsimd.iota(iota_i[:], pattern=[[1, NS]], base=0, channel_multiplier=0)
            iotaf = consts.tile([128, NS], FP32)
            nc.vector.tensor_copy(out=iotaf[:], in_=iota_i[:])

            # block-diagonal embed table: 8 blocks of [8 species, 64 feats]
            blkdiag = consts.tile([64, 8 * F], FP32)
            nc.vector.memset(blkdiag[:], 0.0)
            for s in range(8):
                nc.sync.dma_start(
                    out=blkdiag[8 * s : 8 * s + 8, F * s : F * s + F],
                    in_=etab.ap(),
                )
            etab_sb = consts.tile([NS, F], FP32)
            nc.sync.dma_start(out=etab_sb[:], in_=etab.ap())

            for t in range(nt):
                cs = slice(t * P, (t + 1) * P)

                # ---- load indices / distances ----
                idx = io.tile([P, 17], I32)
                nc.sync.dma_start(out=idx[:], in_=ij_idx.ap()[cs, :])
                d = io.tile([P, N], FP32)
                nc.sync.dma_start(out=d[:], in_=dist.ap()[cs, :])

                # ---- gather xyz rows and species ids ----
                # HW indirect DMA semantics: ONE index per partition, reading a
                # contiguous block per index -> one gather per slot.
                gx = io.tile([P, 17 * 3], FP32)
                sp = io.tile([P, 17], I32)
                for s in range(17):
                    nc.gpsimd.indirect_dma_start(
                        out=gx[:, 3 * s : 3 * s + 3],
                        out_offset=None,
                        in_=axyz.ap(),
                        in_offset=bass.IndirectOffsetOnAxis(
                            ap=idx[:, s : s + 1], axis=0
                        ),
                    )
                    nc.gpsimd.indirect_dma_start(
                        out=sp[:, s : s + 1],
                        out_offset=None,
                        in_=along.ap(),
                        in_offset=bass.IndirectOffsetOnAxis(
                            ap=idx[:, s : s + 1], axis=0
                        ),
                        element_offset=1,
                    )

                # ---- embeddings: emb(slot0)=emb_i, emb(slot 1+j)=emb_j/d_j ----
                spf = work.tile([P, 17], FP32)
                nc.vector.tensor_copy(out=spf[:], in_=sp[:])
                scale = work.tile([P, 17], FP32)
                nc.gpsimd.memset(scale[:, 0:1], 1.0)
                nc.vector.reciprocal(out=scale[:, 1:17], in_=d[:])

                onehot = work.tile([P, 17 * NS], FP32)
                oh3 = onehot[:].rearrange("p (s e) -> p s e", e=NS)
                nc.vector.tensor_tensor(
                    out=oh3,
                    in0=spf[:].unsqueeze(2).to_broadcast([P, 17, NS]),
                    in1=iotaf[:P, :].unsqueeze(1).to_broadcast([P, 17, NS]),
                    op=ALU.is_equal,
                )
                ohs = work.tile([P, 17 * NS], FP32)
                nc.vector.tensor_tensor(
                    out=ohs[:].rearrange("p (s e) -> p s e", e=NS),
                    in0=oh3,
                    in1=scale[:].unsqueeze(2).to_broadcast([P, 17, NS]),
                    op=ALU.mult,
                )

                embdiv = work.tile([P, 17 * F], FP32)
                # groups: slots 0..7 and 8..15 via block-diag matmul; slot 16 alone
                for g in range(2):
                    trp = psp.tile([64, P], FP32, tag="trp")
                    nc.tensor.transpose(
                        out=trp[:],
                        in_=ohs[:, 64 * g : 64 * (g + 1)],
                        identity=ident[:P, :P],
                    )
                    trs = work.tile([64, P], FP32, tag="trs")
                    nc.vector.tensor_copy(out=trs[:], in_=trp[:])
                    mm = psp.tile([P, 8 * F], FP32, tag="mm")
                    nc.tensor.matmul(
                        out=mm[:], lhsT=trs[:], rhs=blkdiag[:], start=True, stop=True
                    )
                    nc.vector.tensor_copy(
                        out=embdiv[:, 8 * F * g : 8 * F * (g + 1)], in_=mm[:]
                    )
                trp2 = psp.tile([8, P], FP32, tag="trp2")
                nc.tensor.transpose(
                    out=trp2[:], in_=ohs[:, 128:136], identity=ident[:P, :P]
                )
                trs2 = work.tile([8, P], FP32, tag="trs2")
                nc.vector.tensor_copy(out=trs2[:], in_=trp2[:])
                mm2 = psp.tile([P, F], FP32, tag="mm2")
                nc.tensor.matmul(
                    out=mm2[:], lhsT=trs2[:], rhs=etab_sb[:], start=True, stop=True
                )
                nc.vector.tensor_copy(out=embdiv[:, 16 * F : 17 * F], in_=mm2[:])

                # ---- geometry: vec, pairwise dot, angles ----
                gx3 = gx[:].rearrange("p (s e) -> p s e", e=3)
                vec = work.tile([P, N * 3], FP32)
                v3 = vec[:].rearrange("p (s e) -> p s e", e=3)
                nc.vector.tensor_tensor(
                    out=v3,
                    in0=gx3[:, 1:17, :],
                    in1=gx3[:, 0:1, :].to_broadcast([P, N, 3]),
                    op=ALU.subtract,
                )

                dot = work.tile([P, N * N], FP32)
                dot3 = dot[:].rearrange("p (j k) -> p j k", k=N)
                tmp = work.tile([P, N * N], FP32)
                tmp3 = tmp[:].rearrange("p (j k) -> p j k", k=N)
                for e in range(3):
                    vj = v3[:, :, e].unsqueeze(2).to_broadcast([P, N, N])
                    vk = v3[:, :, e].unsqueeze(1).to_broadcast([P, N, N])
                    if e == 0:
                        nc.vector.tensor_tensor(out=dot3, in0=vj, in1=vk, op=ALU.mult)
                    else:
                        nc.vector.tensor_tensor(out=tmp3, in0=vj, in1=vk, op=ALU.mult)
                        nc.vector.tensor_tensor(
                            out=dot3, in0=dot3, in1=tmp3, op=ALU.add
                        )

                n2 = work.tile([P, N], FP32)
                nc.vector.tensor_scalar_max(
                    out=n2[:], in0=dot[:, 0 : N * N : N + 1], scalar1=COS_EPS * COS_EPS
                )
                nrm = work.tile([P, N], FP32)
                nc.scalar.sqrt(nrm[:], n2[:])
                rinv = work.tile([P, N], FP32)
                nc.vector.reciprocal(out=rinv[:], in_=nrm[:])

                cosb = work.tile([P, N * N], FP32)
                cosb3 = cosb[:].rearrange("p (j k) -> p j k", k=N)
                nc.vector.tensor_tensor(
                    out=cosb3,
                    in0=dot3,
                    in1=rinv[:].unsqueeze(2).to_broadcast([P, N, N]),
                    op=ALU.mult,
                )
                x = work.tile([P, N * N], FP32)
                nc.vector.scalar_tensor_tensor(
                    out=x[:].rearrange("p (j k) -> p j k", k=N),
                    in0=cosb3,
                    scalar=CLIP,
                    in1=rinv[:].unsqueeze(1).to_broadcast([P, N, N]),
                    op0=ALU.mult,
                    op1=ALU.mult,
                )
                # arccos via A&S-style polynomial:
                #   arccos(|x|) = sqrt(1-|x|) * P7(|x|), arccos(x) = pi - arccos(-x)
                ACOS_C = [
                    1.5707963050, -0.2145988016, 0.0889789874, -0.0501743046,
                    0.0308918810, -0.0170881256, 0.0066700901, -0.0012624911,
                ]
                ax = work.tile([P, N * N], FP32)
                nc.vector.scalar_tensor_tensor(
                    out=ax[:], in0=x[:], scalar=-1.0, in1=x[:],
                    op0=ALU.mult, op1=ALU.max,
                )
                omx = work.tile([P, N * N], FP32)
                nc.vector.tensor_scalar(
                    out=omx[:], in0=ax[:], scalar1=-1.0, scalar2=1.0,
                    op0=ALU.mult, op1=ALU.add,
                )
                sq1 = work.tile([P, N * N], FP32)
                nc.scalar.sqrt(sq1[:], omx[:])
                hcur = work.tile([P, N * N], FP32, tag="horner_a")
                nc.vector.tensor_scalar_mul(out=hcur[:], in0=ax[:], scalar1=ACOS_C[7])
                for ci, cval in enumerate([ACOS_C[i] for i in range(6, 0, -1)]):
                    hnxt = work.tile(
                        [P, N * N], FP32, tag=f"horner_{'b' if ci % 2 == 0 else 'a'}"
                    )
                    nc.vector.scalar_tensor_tensor(
                        out=hnxt[:], in0=hcur[:], scalar=cval, in1=ax[:],
                        op0=ALU.add, op1=ALU.mult,
                    )
                    hcur = hnxt
                respos = work.tile([P, N * N], FP32)
                nc.vector.scalar_tensor_tensor(
                    out=respos[:], in0=hcur[:], scalar=ACOS_C[0], in1=sq1[:],
                    op0=ALU.add, op1=ALU.mult,
                )
                sgn = work.tile([P, N * N], FP32)
                nc.scalar.sign(sgn[:], x[:])
                angm = work.tile([P, N * N], FP32)
                nc.vector.scalar_tensor_tensor(
                    out=angm[:], in0=respos[:], scalar=float(-np.pi / 2), in1=sgn[:],
                    op0=ALU.add, op1=ALU.mult,
                )
                ang = work.tile([P, N * N], FP32)
                nc.vector.tensor_scalar_add(
                    out=ang[:], in0=angm[:], scalar1=float(np.pi / 2)
                )
                ang3 = ang[:].rearrange("p (j k) -> p j k", k=N)

                # ---- assemble output rows (full 16x16 grid incl. diagonal) ----
                # All copies for one chunk go on ONE engine (alternating
                # DVE/ACT per chunk): a DMACopy can only carry a single sync
                # wait, so each out-DMA must depend on one engine clock only.
                for aa in range(4):  # chunks of 4 j's
                    cp_fn = (
                        nc.vector.tensor_copy if aa % 2 == 0 else nc.scalar.copy
                    )
                    och = ochp.tile([P, 4 * N * ROW], FP32, tag="och")
                    ov = och[:].rearrange("p (m k r) -> p m k r", k=N, r=ROW)
                    for m in range(4):
                        j = 4 * aa + m
                        cp_fn(
                            out=ov[:, m, :, 0:1],
                            in_=d[:, j : j + 1].unsqueeze(2).to_broadcast([P, N, 1]),
                        )
                        cp_fn(out=ov[:, m, :, 1:2], in_=d[:].unsqueeze(2))
                        cp_fn(out=ov[:, m, :, 2:3], in_=ang3[:, j, :].unsqueeze(2))
                        cp_fn(
                            out=ov[:, m, :, 3 : 3 + F],
                            in_=embdiv[:, 0:F].unsqueeze(1).to_broadcast([P, N, F]),
                        )
                        cp_fn(
                            out=ov[:, m, :, 3 + F : 3 + 2 * F],
                            in_=embdiv[:, (1 + j) * F : (2 + j) * F]
                            .unsqueeze(1)
                            .to_broadcast([P, N, F]),
                        )
                        cp_fn(
                            out=ov[:, m, :, 3 + 2 * F : 3 + 3 * F],
                            in_=embdiv[:, F : 17 * F].rearrange(
                                "p (k f) -> p k f", f=F
                            ),
                        )

                    # DMA out: skip diagonal rows (local row 4a+17m, m=0..3)
                    j0 = 4 * aa
                    p0 = 60 * aa  # valid rows before this chunk
                    # S0: local rows [0, j0)
                    if j0 > 0:
                        nc.sync.dma_start(
                            out=od[cs, p0 * ROW : (p0 + j0) * ROW],
                            in_=och[:, 0 : j0 * ROW],
                        )
                    # S1: local rows (j0, j0+52) as 3 x 17 keeping 16 each
                    src = och[:, (j0 + 1) * ROW : (j0 + 52) * ROW].rearrange(
                        "p (x q) -> p x q", x=3
                    )[:, :, 0 : 16 * ROW]
                    nc.sync.dma_start(
                        out=od[cs, (p0 + j0) * ROW : (p0 + j0 + 48) * ROW],
                        in_=src,
                    )
                    # S2: local rows [j0+52, 64)
                    if j0 < 12:
                        nc.sync.dma_start(
                            out=od[cs, (p0 + j0 + 48) * ROW : (p0 + 60) * ROW],
                            in_=och[:, (j0 + 52) * ROW : 64 * ROW],
                        )

    nc.compile()
    return nc


_NC_CACHE = None


def _get_nc() -> bass.Bass:
    global _NC_CACHE
    if _NC_CACHE is None:
        _NC_CACHE = build_program()
    return _NC_CACHE


def _as_i32(x: np.ndarray) -> np.ndarray:
    return np.ascontiguousarray(np.asarray(x).astype(np.int32, copy=False))


def _as_f32(x: np.ndarray) -> np.ndarray:
    return np.ascontiguousarray(np.asarray(x).astype(np.float32, copy=False))


def make_in_maps(atom_i_idx, atom_j_idx, dist_ij, atoms_xyz, atoms_long, embed_table):
    i_full = _as_i32(atom_i_idx)
    j_full = _as_i32(atom_j_idx)
    d_full = _as_f32(dist_ij)
    axyz = _as_f32(atoms_xyz)
    along = _as_i32(atoms_long)
    etab = _as_f32(embed_table)
    ij_full = np.ascontiguousarray(
        np.concatenate([i_full[:, None], j_full], axis=1)
    )
    in_maps = []
    for r in range(N_CORES):
        rs = slice(r * CP, (r + 1) * CP)
        in_maps.append(
            {
                "ij_idx": np.ascontiguousarray(ij_full[rs]),
                "dist": np.ascontiguousarray(d_full[rs]),
                "axyz": axyz,
                "along": along,
                "etab": etab,
            }
        )
    return in_maps


def kernel(
    nNeigh=None,
    atom_i_idx=None,
    atom_j_idx=None,
    dist_ij=None,
    atoms_xyz=None,
    atoms_long=None,
    embed_table=None,
    **_unused,
):
    in_maps = make_in_maps(
        atom_i_idx, atom_j_idx, dist_ij, atoms_xyz, atoms_long, embed_table
    )
    res = run_bass_kernel_spmd(_get_nc(), in_maps, list(range(N_CORES)))
    outs = [res.results[r]["out"].reshape(CP, NPAIR, ROW) for r in range(N_CORES)]
    ang_desc = np.concatenate(outs, axis=0)
    return np.asarray(atom_i_idx).reshape(-1), ang_desc


# revision 22
# speedup vs baseline: 1.1241x; 1.1241x over previous
"""Trainium2 Bass kernel for ComputeRealAngleInput (gnn angle descriptor).

Contract: kernel(**inputs) takes the FULL unsharded inputs (as produced by
setup_inputs()) and returns the full output tuple
(atom_i_idx.reshape(-1), ang_desc[C, N*(N-1), 3+3F]).

Strategy (data-parallel over center atoms, 8 NeuronCores):
  - shard atom_i_idx / atom_j_idx / dist_ij along dim 0 (500 centers/core)
  - replicate atoms_xyz / atoms_long / embed_table on every core
  - per core: 4 tiles of 125 centers (partition dim = centers)
      * indirect-DMA gather of xyz rows + species ids (17 per center)
      * embeddings via one-hot(species)*scale -> PE transpose -> matmul
        with a block-diagonal embed table (computes emb/d for all slots)
      * pairwise angles via broadcast-AP vector ops;
        arccos(x) = pi/2 - arctan(x * rsqrt(1 - x^2))
      * output assembled in SBUF as the full 16x16 (j,k) grid; the DMA to
        DRAM skips the diagonal rows with the flat[1:].reshape(15,17)[:,:16]
        access-pattern trick so exactly 240 rows per center are written.
"""

import numpy as np

import concourse.bass as bass
import concourse.bacc as bacc
import concourse.mybir as mybir
from concourse import tile
from concourse.bass_utils import run_bass_kernel_spmd
from concourse.masks import make_identity

# ---- problem constants (hardcoded; must match the reference problem) ----
N_CORES = 8
C_FULL = 4000
CP = C_FULL // N_CORES  # 500 centers per core
P = 125                 # centers per tile (partition dim)
NT = CP // P            # 4 center tiles per core
N = 16                  # neighbours
F = 64                  # embedding features
NA = 20000              # atoms
NS = 8                  # species
ROW = 3 + 3 * F         # 195 output columns
NPAIR = N * (N - 1)     # 240 off-diagonal (j,k) pairs
COS_EPS = 1e-8
CLIP = 0.9999

FP32 = mybir.dt.float32
I32 = mybir.dt.int32
AF = mybir.ActivationFunctionType
ALU = mybir.AluOpType


REC_W = 64  # padded per-atom record: [x, y, z, species_f32, 0...] (256B rows)
NIDX = 17 * 128  # gather indices per center tile (slot-major, 128-padded)


def build_program(cp: int = CP) -> bass.Bass:
    nt = cp // P
    nc = bacc.Bacc("TRN2", target_bir_lowering=False, debug=False)

    idx16 = nc.dram_tensor("idx16", [nt, 128, NIDX // 16], mybir.dt.int16,
                           kind="ExternalInput")
    dist = nc.dram_tensor("dist", [cp, N], FP32, kind="ExternalInput")
    rec = nc.dram_tensor("rec", [NA, REC_W], FP32, kind="ExternalInput")
    etab = nc.dram_tensor("etab", [NS, F], FP32, kind="ExternalInput")
    out = nc.dram_tensor("out", [cp, NPAIR * ROW], FP32, kind="ExternalOutput")

    od = out.ap()

    with tile.TileContext(nc) as tc:
        with (
            tc.tile_pool(name="consts", bufs=1) as consts,
            tc.tile_pool(name="io", bufs=4) as io,
            tc.tile_pool(name="work", bufs=2) as work,
            tc.tile_pool(name="ochp", bufs=2) as ochp,
            tc.tile_pool(name="psp", bufs=2, space="PSUM") as psp,
        ):
            # ---- one-time constants ----
            ident = consts.tile([128, 128], FP32)
            make_identity(nc, ident[:])

            iota_i = consts.tile([128, NS], I32)
            nc.gpsimd.iota(iota_i[:], pattern=[[1, NS]], base=0, channel_multiplier=0)
            iotaf = consts.tile([128, NS], FP32)
            nc.vector.tensor_copy(out=iotaf[:], in_=iota_i[:])

            # block-diagonal embed table: 8 blocks of [8 species, 64 feats]
            blkdiag = consts.tile([64, 8 * F], FP32)
            nc.vector.memset(blkdiag[:], 0.0)
            for s in range(8):
                nc.sync.dma_start(
                    out=blkdiag[8 * s : 8 * s + 8, F * s : F * s + F],
                    in_=etab.ap(),
                )
            etab_sb = consts.tile([NS, F], FP32)
            nc.sync.dma_start(out=etab_sb[:], in_=etab.ap())

            for t in range(nt):
                cs = slice(t * P, (t + 1) * P)

                # ---- load indices / distances ----
                idxs = io.tile([128, NIDX // 16], mybir.dt.int16)
                nc.sync.dma_start(out=idxs[:], in_=idx16.ap()[t])
                d = io.tile([P, N], FP32)
                nc.sync.dma_start(out=d[:], in_=dist.ap()[cs, :])

                # ---- gather per-atom records (xyz + species) in ONE DMA ----
                # grec[p, slot, :] = rec[idx[slot*128 + p], :]
                grec = io.tile([128, 17 * REC_W], FP32)
                nc.gpsimd.dma_gather(
                    out_ap=grec[:].rearrange("p (s e) -> p s e", e=REC_W),
                    in_ap=rec.ap(),
                    idxs_ap=idxs[:],
                    num_idxs=NIDX,
                    num_idxs_reg=NIDX,
                    elem_size=REC_W,
                    single_packet=False,
                )
                g3 = grec[:P].rearrange("p (s e) -> p s e", e=REC_W)
                spf = g3[:, :, 3]  # species as f32, [P, 17]

                # ---- embeddings: emb(slot0)=emb_i, emb(slot 1+j)=emb_j/d_j ----
                scale = work.tile([P, 17], FP32)
                nc.gpsimd.memset(scale[:, 0:1], 1.0)
                nc.vector.reciprocal(out=scale[:, 1:17], in_=d[:])

                onehot = work.tile([P, 17 * NS], FP32)
                oh3 = onehot[:].rearrange("p (s e) -> p s e", e=NS)
                nc.vector.tensor_tensor(
                    out=oh3,
                    in0=spf.unsqueeze(2).to_broadcast([P, 17, NS]),
                    in1=iotaf[:P, :].unsqueeze(1).to_broadcast([P, 17, NS]),
                    op=ALU.is_equal,
                )
                ohs = work.tile([P, 17 * NS], FP32)
                nc.vector.tensor_tensor(
                    out=ohs[:].rearrange("p (s e) -> p s e", e=NS),
                    in0=oh3,
                    in1=scale[:].unsqueeze(2).to_broadcast([P, 17, NS]),
                    op=ALU.mult,
                )

                embdiv = work.tile([P, 17 * F], FP32)
                # groups: slots 0..7 and 8..15 via block-diag matmul; slot 16 alone
                for g in range(2):
                    trp = psp.tile([64, P], FP32, tag="trp")
                    nc.tensor.transpose(
                        out=trp[:],
                        in_=ohs[:, 64 * g : 64 * (g + 1)],
                        identity=ident[:P, :P],
                    )
                    trs = work.tile([64, P], FP32, tag="trs")
                    nc.vector.tensor_copy(out=trs[:], in_=trp[:])
                    mm = psp.tile([P, 8 * F], FP32, tag="mm")
                    nc.tensor.matmul(
                        out=mm[:], lhsT=trs[:], rhs=blkdiag[:], start=True, stop=True
                    )
                    nc.vector.tensor_copy(
                        out=embdiv[:, 8 * F * g : 8 * F * (g + 1)], in_=mm[:]
                    )
                trp2 = psp.tile([8, P], FP32, tag="trp2")
                nc.tensor.transpose(
                    out=trp2[:], in_=ohs[:, 128:136], identity=ident[:P, :P]
                )
                trs2 = work.tile([8, P], FP32, tag="trs2")
                nc.vector.tensor_copy(out=trs2[:], in_=trp2[:])
                mm2 = psp.tile([P, F], FP32, tag="mm2")
                nc.tensor.matmul(
                    out=mm2[:], lhsT=trs2[:], rhs=etab_sb[:], start=True, stop=True
                )
                nc.vector.tensor_copy(out=embdiv[:, 16 * F : 17 * F], in_=mm2[:])

                # ---- geometry: vec, pairwise dot, angles ----
                vec = work.tile([P, N * 3], FP32)
                v3 = vec[:].rearrange("p (s e) -> p s e", e=3)
                nc.vector.tensor_tensor(
                    out=v3,
                    in0=g3[:, 1:17, 0:3],
                    in1=g3[:, 0:1, 0:3].to_broadcast([P, N, 3]),
                    op=ALU.subtract,
                )

                dot = work.tile([P, N * N], FP32)
                dot3 = dot[:].rearrange("p (j k) -> p j k", k=N)
                tmp = work.tile([P, N * N], FP32)
                tmp3 = tmp[:].rearrange("p (j k) -> p j k", k=N)
                for e in range(3):
                    vj = v3[:, :, e].unsqueeze(2).to_broadcast([P, N, N])
                    vk = v3[:, :, e].unsqueeze(1).to_broadcast([P, N, N])
                    if e == 0:
                        nc.vector.tensor_tensor(out=dot3, in0=vj, in1=vk, op=ALU.mult)
                    else:
                        nc.vector.tensor_tensor(out=tmp3, in0=vj, in1=vk, op=ALU.mult)
                        nc.vector.tensor_tensor(
                            out=dot3, in0=dot3, in1=tmp3, op=ALU.add
                        )

                n2 = work.tile([P, N], FP32)
                nc.vector.tensor_scalar_max(
                    out=n2[:], in0=dot[:, 0 : N * N : N + 1], scalar1=COS_EPS * COS_EPS
                )
                nrm = work.tile([P, N], FP32)
                nc.scalar.sqrt(nrm[:], n2[:])
                rinv = work.tile([P, N], FP32)
                nc.vector.reciprocal(out=rinv[:], in_=nrm[:])

                cosb = work.tile([P, N * N], FP32)
                cosb3 = cosb[:].rearrange("p (j k) -> p j k", k=N)
                nc.vector.tensor_tensor(
                    out=cosb3,
                    in0=dot3,
                    in1=rinv[:].unsqueeze(2).to_broadcast([P, N, N]),
                    op=ALU.mult,
                )
                x = work.tile([P, N * N], FP32)
                nc.vector.scalar_tensor_tensor(
                    out=x[:].rearrange("p (j k) -> p j k", k=N),
                    in0=cosb3,
                    scalar=CLIP,
                    in1=rinv[:].unsqueeze(1).to_broadcast([P, N, N]),
                    op0=ALU.mult,
                    op1=ALU.mult,
                )
                # arccos via A&S-style polynomial:
                #   arccos(|x|) = sqrt(1-|x|) * P7(|x|), arccos(x) = pi - arccos(-x)
                ACOS_C = [
                    1.5707963050, -0.2145988016, 0.0889789874, -0.0501743046,
                    0.0308918810, -0.0170881256, 0.0066700901, -0.0012624911,
                ]
                ax = work.tile([P, N * N], FP32)
                nc.vector.scalar_tensor_tensor(
                    out=ax[:], in0=x[:], scalar=-1.0, in1=x[:],
                    op0=ALU.mult, op1=ALU.max,
                )
                omx = work.tile([P, N * N], FP32)
                nc.vector.tensor_scalar(
                    out=omx[:], in0=ax[:], scalar1=-1.0, scalar2=1.0,
                    op0=ALU.mult, op1=ALU.add,
                )
                sq1 = work.tile([P, N * N], FP32)
                nc.scalar.sqrt(sq1[:], omx[:])
                hcur = work.tile([P, N * N], FP32, tag="horner_a")
                nc.vector.tensor_scalar_mul(out=hcur[:], in0=ax[:], scalar1=ACOS_C[7])
                for ci, cval in enumerate([ACOS_C[i] for i in range(6, 0, -1)]):
                    hnxt = work.tile(
                        [P, N * N], FP32, tag=f"horner_{'b' if ci % 2 == 0 else 'a'}"
                    )
                    nc.vector.scalar_tensor_tensor(
                        out=hnxt[:], in0=hcur[:], scalar=cval, in1=ax[:],
                        op0=ALU.add, op1=ALU.mult,
                    )
                    hcur = hnxt
                respos = work.tile([P, N * N], FP32)
                nc.vector.scalar_tensor_tensor(
                    out=respos[:], in0=hcur[:], scalar=ACOS_C[0], in1=sq1[:],
                    op0=ALU.add, op1=ALU.mult,
                )
                sgn = work.tile([P, N * N], FP32)
                nc.scalar.sign(sgn[:], x[:])
                angm = work.tile([P, N * N], FP32)
                nc.vector.scalar_tensor_tensor(
                    out=angm[:], in0=respos[:], scalar=float(-np.pi / 2), in1=sgn[:],
                    op0=ALU.add, op1=ALU.mult,
                )
                ang = work.tile([P, N * N], FP32)
                nc.vector.tensor_scalar_add(
                    out=ang[:], in0=angm[:], scalar1=float(np.pi / 2)
                )
                ang3 = ang[:].rearrange("p (j k) -> p j k", k=N)

                # ---- assemble output rows (diagonal skipped in SBUF) ----
                # Each chunk holds 4 j-blocks of 15 valid rows -> the out-DMA
                # is a single fully-contiguous [P, 60*ROW] transfer.
                KR = N - 1  # 15 valid k-rows per j
                for aa in range(4):  # chunks of 4 j's
                    och = ochp.tile([P, 4 * KR * ROW], FP32, tag="och")
                    ov = och[:].rearrange("p (m k r) -> p m k r", k=KR, r=ROW)
                    for m in range(4):
                        j = 4 * aa + m
                        # tiny columns (d_j, d_k, ang) on DVE
                        nc.vector.tensor_copy(
                            out=ov[:, m, :, 0:1],
                            in_=d[:, j : j + 1].unsqueeze(2).to_broadcast([P, KR, 1]),
                        )
                        if j > 0:
                            nc.vector.tensor_copy(
                                out=ov[:, m, 0:j, 1:2], in_=d[:, 0:j].unsqueeze(2)
                            )
                            nc.vector.tensor_copy(
                                out=ov[:, m, 0:j, 2:3],
                                in_=ang3[:, j, 0:j].unsqueeze(2),
                            )
                        if j < N - 1:
                            nc.vector.tensor_copy(
                                out=ov[:, m, j:KR, 1:2],
                                in_=d[:, j + 1 : N].unsqueeze(2),
                            )
                            nc.vector.tensor_copy(
                                out=ov[:, m, j:KR, 2:3],
                                in_=ang3[:, j, j + 1 : N].unsqueeze(2),
                            )
                        # emb_i broadcast on ACT
                        nc.scalar.copy(
                            out=ov[:, m, :, 3 : 3 + F],
                            in_=embdiv[:, 0:F].unsqueeze(1).to_broadcast([P, KR, F]),
                        )
                        # emb_j/d_j broadcast on GpSimd
                        nc.gpsimd.tensor_copy(
                            out=ov[:, m, :, 3 + F : 3 + 2 * F],
                            in_=embdiv[:, (1 + j) * F : (2 + j) * F]
                            .unsqueeze(1)
                            .to_broadcast([P, KR, F]),
                        )
                        # emb_k/d_k (k != j) on DVE
                        if j > 0:
                            nc.vector.tensor_copy(
                                out=ov[:, m, 0:j, 3 + 2 * F : 3 + 3 * F],
                                in_=embdiv[:, F : (1 + j) * F].rearrange(
                                    "p (k f) -> p k f", f=F
                                ),
                            )
                        if j < N - 1:
                            nc.vector.tensor_copy(
                                out=ov[:, m, j:KR, 3 + 2 * F : 3 + 3 * F],
                                in_=embdiv[:, (2 + j) * F : 17 * F].rearrange(
                                    "p (k f) -> p k f", f=F
                                ),
                            )

                    # one contiguous DMA per chunk
                    p0 = 60 * aa
                    nc.sync.dma_start(
                        out=od[cs, p0 * ROW : (p0 + 60) * ROW],
                        in_=och[:],
                    )

    nc.compile()
    return nc


_NC_CACHE = None


def _get_nc() -> bass.Bass:
    global _NC_CACHE
    if _NC_CACHE is None:
        _NC_CACHE = build_program()
    return _NC_CACHE


def _as_i32(x: np.ndarray) -> np.ndarray:
    return np.ascontiguousarray(np.asarray(x).astype(np.int32, copy=False))


def _as_f32(x: np.ndarray) -> np.ndarray:
    return np.ascontiguousarray(np.asarray(x).astype(np.float32, copy=False))


def make_in_maps(atom_i_idx, atom_j_idx, dist_ij, atoms_xyz, atoms_long, embed_table):
    i_full = _as_i32(atom_i_idx)
    j_full = _as_i32(atom_j_idx)
    d_full = _as_f32(dist_ij)
    axyz = _as_f32(atoms_xyz)
    along = _as_i32(atoms_long)
    etab = _as_f32(embed_table)
    ij_full = np.concatenate([i_full[:, None], j_full], axis=1)  # [C, 17]

    # packed per-atom record table: [x, y, z, species] padded to 256B rows
    rec = np.zeros((NA, REC_W), dtype=np.float32)
    rec[:, 0:3] = axyz
    rec[:, 3] = along[:, 1].astype(np.float32)

    # dma_gather index layout: unwrapped position g = slot*128 + p holds
    # ij[tile_center p, slot]; idx16[t, g%16, g//16] = that, replicated
    # across the 8 gpsimd cores (partition groups of 16).
    nt = CP // P
    g = np.arange(NIDX)
    slot, p = g // 128, g % 128
    in_maps = []
    for r in range(N_CORES):
        rs = slice(r * CP, (r + 1) * CP)
        ijr = ij_full[rs].reshape(nt, P, 17)
        idx16 = np.zeros((nt, 128, NIDX // 16), dtype=np.int16)
        for t in range(nt):
            vals = np.where(p < P, ijr[t, np.minimum(p, P - 1), slot], 0)
            wrapped = np.zeros((16, NIDX // 16), dtype=np.int16)
            wrapped[g % 16, g // 16] = vals
            idx16[t] = np.tile(wrapped, (8, 1))
        in_maps.append(
            {
                "idx16": idx16,
                "dist": np.ascontiguousarray(d_full[rs]),
                "rec": rec,
                "etab": etab,
            }
        )
    return in_maps


def kernel(
    nNeigh=None,
    atom_i_idx=None,
    atom_j_idx=None,
    dist_ij=None,
    atoms_xyz=None,
    atoms_long=None,
    embed_table=None,
    **_unused,
):
    in_maps = make_in_maps(
        atom_i_idx, atom_j_idx, dist_ij, atoms_xyz, atoms_long, embed_table
    )
    res = run_bass_kernel_spmd(_get_nc(), in_maps, list(range(N_CORES)))
    outs = [res.results[r]["out"].reshape(CP, NPAIR, ROW) for r in range(N_CORES)]
    ang_desc = np.concatenate(outs, axis=0)
    return np.asarray(atom_i_idx).reshape(-1), ang_desc
